# revision 1
# baseline (speedup 1.0000x reference)
"""Trainium2 Bass kernel for the attention-gate block.

Math (per sample n, after folding BN into the convs):
  X     = x[n, :, ::2, ::2].reshape(C, 4)                 # C=512, L=4
  act_k = relu(Wk' @ X + bk')            k=0,1,2          # D=64 each
  S     = act0^T act1  (4x4);  P = softmax_rows(S)
  Z     = P @ act2^T  (4x64)
  Y     = W4' @ Z^T + b4'                                  # (512, 4)
  out[n,c,h,w] = x[n,c,h,w] + Y[c,h]                       # broadcast over w

Device mapping (per core, 256 samples, blocks of 64):
  - channel packing c = 4p + j (p = partition, j = 0..3): each
    (partition, sample) moves one 256B-contiguous run, so a block is ONE
    big DMA each way (loads on the sync HWDGE queue, stores on scalar's).
    Weights are permuted host-side to match, so compute is unchanged.
  - GEMM1 computes q and k over 4 contraction groups; v is computed
    directly transposed ([samples*4 parts, d]) by swapping matmul
    operands, with its bias folded in via a K=1 ones-row matmul.
  - attention scores for 32 samples at a time come from one [64]x[128,128]
    gram matmul whose block-diagonal 4x4 blocks are the real scores;
    masked exp (ACT + 0/1 block-diag mask on DVE); softmax denominators
    via a ones-column matmul; normalization deferred past the P@V matmul.
  - GEMM2 (BN folded) does the w-broadcast in its rhs AP (step-0 re-read
    of each z column) so the residual add runs on plain stride-1 APs.
"""

import os
import sys

for _p in ("/opt/trn_rl_repo",):
    if _p not in sys.path:
        sys.path.insert(0, _p)

import numpy as np

import concourse.mybir as mybir
from concourse import bacc, tile

EPS = 1e-5
N_TOTAL, C, D, HH, WW = 2048, 512, 64, 4, 4
NCORES = 8
NSH = N_TOTAL // NCORES  # 256 samples per core
BLK = int(os.environ.get("KBLK", "128"))  # samples per block
SUB = 32                 # samples per attention subchunk (4*SUB = 128 cols)
SHIFT = -34.0            # constant exp shift; cancels in the normalization
F32 = mybir.dt.float32

_PROG_CACHE = {}


def build_program(nsh=NSH, blk=BLK, reps=1):
    key = (nsh, blk, reps)
    if key in _PROG_CACHE:
        return _PROG_CACHE[key]

    nc = bacc.Bacc("TRN2", target_bir_lowering=False, debug=False)
    AF = mybir.ActivationFunctionType

    x_in = nc.dram_tensor("x", (nsh, C, HH, WW), F32, kind="ExternalInput")
    wqk = nc.dram_tensor("wqk", (C, 128), F32, kind="ExternalInput")
    bqk = nc.dram_tensor("bqk", (128, 1), F32, kind="ExternalInput")
    w2a = nc.dram_tensor("w2a", (C, D), F32, kind="ExternalInput")
    b2a = nc.dram_tensor("b2a", (1, D), F32, kind="ExternalInput")
    w4t = nc.dram_tensor("w4t", (D, C), F32, kind="ExternalInput")
    b4v = nc.dram_tensor("b4v", (1, C), F32, kind="ExternalInput")
    msk = nc.dram_tensor("msk", (128, 128), F32, kind="ExternalInput")
    out = nc.dram_tensor("out", (nsh, C, HH, WW), F32, kind="ExternalOutput")

    nblk = nsh // blk
    nsub = blk // SUB
    NF = 4 * blk  # free width of a full block of (n, l) columns

    with tile.TileContext(nc) as tc:
        with (
            tc.tile_pool(name="const", bufs=1) as cpool,
            tc.tile_pool(name="xp", bufs=(3 if blk >= 128 else 4)) as xpool,
            tc.tile_pool(name="work", bufs=4) as wpool,
            tc.tile_pool(name="att", bufs=6) as apool,
            tc.tile_pool(name="ps", bufs=6, space="PSUM") as pspool,
            tc.tile_pool(name="psy", bufs=2, space="PSUM") as pypool,
        ):
            wq_sb = cpool.tile([128, 4, D], F32)
            nc.sync.dma_start(
                wq_sb[:], wqk[:, 0:D].rearrange("(k p) d -> p k d", p=128))
            wk_sb = cpool.tile([128, 4, D], F32)
            nc.sync.dma_start(
                wk_sb[:], wqk[:, D:2 * D].rearrange("(k p) d -> p k d", p=128))
            bq_sb = cpool.tile([D, 1], F32)
            nc.sync.dma_start(bq_sb[:], bqk[0:D])
            bk_sb = cpool.tile([D, 1], F32)
            nc.sync.dma_start(bk_sb[:], bqk[D:2 * D])
            w2a_sb = cpool.tile([128, 4, D], F32)
            nc.sync.dma_start(w2a_sb[:], w2a[:].rearrange("(k p) d -> p k d", p=128))
            b2a_sb = cpool.tile([1, D], F32)
            nc.sync.dma_start(b2a_sb[:], b2a[:])
            w4t_sb = cpool.tile([D, 4, 128], F32)
            nc.sync.dma_start(w4t_sb[:], w4t[:].rearrange("d (k p) -> d k p", p=128))
            b4c_sb = cpool.tile([128, 4], F32)
            nc.sync.dma_start(
                b4c_sb[:], b4v[:].rearrange("x (j p) -> p (x j)", j=4))
            msk_sb = cpool.tile([128, 128], F32)
            nc.sync.dma_start(msk_sb[:], msk[:])
            ones_sb = cpool.tile([1, max(NF, 512)], F32)
            nc.vector.memset(ones_sb[:], 1.0)
            ones_col = cpool.tile([128, 1], F32)
            nc.vector.memset(ones_col[:], 1.0)
            shift_sb = cpool.tile([128, 1], F32)
            nc.vector.memset(shift_sb[:], SHIFT)

            # channel packing c = 4p + j: one DMA per block each way
            xv = x_in[:].rearrange("(b n) (p j) h w -> b p n (j h w)", j=4, n=blk)
            ov = out[:].rearrange("(b n) (p j) h w -> b p n (j h w)", j=4, n=blk)

            for b in [b for _ in range(reps) for b in range(nblk)]:
                x_t = xpool.tile([128, blk, 64], F32, tag="x")
                nc.sync.dma_start(x_t[:], xv[b])
                xtv = x_t[:].rearrange("p n (j h w) -> p n j h w", j=4, h=4)

                # gather the ::2,::2 columns -> [128, j, n, l] with l=(h',w')
                xr = wpool.tile([128, 4, blk, 4], F32, tag="xr")
                nc.vector.tensor_copy(
                    xr[:].rearrange("p j n (a c) -> p j n a c", a=2),
                    xtv[:, :, :, 0:4:2, 0:4:2].transpose([0, 2, 1, 3, 4]),
                )
                xrf = xr[:].rearrange("p j n l -> p j (n l)")

                # GEMM1 q and k: [c=512 contraction] -> psum [64, NF] each
                ps_q = pspool.tile([D, NF], F32, tag="ps")
                ps_k = pspool.tile([D, NF], F32, tag="ps")
                for j in range(4):
                    nc.tensor.matmul(
                        ps_q[:], lhsT=wq_sb[:, j], rhs=xrf[:, j],
                        start=(j == 0), stop=(j == 3),
                    )
                for j in range(4):
                    nc.tensor.matmul(
                        ps_k[:], lhsT=wk_sb[:, j], rhs=xrf[:, j],
                        start=(j == 0), stop=(j == 3),
                    )
                a_q = wpool.tile([D, NF], F32, tag="aq")
                nc.scalar.activation(a_q[:], ps_q[:], AF.Relu, bias=bq_sb[:])
                a_k = wpool.tile([D, NF], F32, tag="ak")
                nc.scalar.activation(a_k[:], ps_k[:], AF.Relu, bias=bk_sb[:])

                # phase 1: independent PE work for all subchunks
                ph_vt, ph_g = [], []
                for s in range(nsub):
                    cl = slice(s * 128, s * 128 + 128)
                    ps_vt = pspool.tile([128, D], F32, tag="ps")
                    for j in range(4):
                        nc.tensor.matmul(
                            ps_vt[:], lhsT=xrf[:, j, cl], rhs=w2a_sb[:, j],
                            start=(j == 0), stop=False,
                        )
                    nc.tensor.matmul(
                        ps_vt[:], lhsT=ones_sb[:, 0:128], rhs=b2a_sb[:],
                        start=False, stop=True,
                    )
                    ps_g = pspool.tile([128, 128], F32, tag="ps")
                    nc.tensor.matmul(
                        ps_g[:], lhsT=a_k[:, cl], rhs=a_q[:, cl],
                        start=True, stop=True,
                    )
                    ph_vt.append(ps_vt)
                    ph_g.append(ps_g)
                # phase 2: ACT/DVE consumers for all subchunks
                ph_a2t, ph_p0 = [], []
                for s in range(nsub):
                    a2t = apool.tile([128, D], F32, tag="a2t")
                    nc.scalar.activation(a2t[:], ph_vt[s][:], AF.Relu)
                    e_t = apool.tile([128, 128], F32, tag="e")
                    nc.scalar.activation(e_t[:], ph_g[s][:], AF.Exp,
                                         bias=shift_sb[:])
                    p0 = apool.tile([128, 128], F32, tag="p0")
                    nc.vector.tensor_mul(p0[:], e_t[:], msk_sb[:])
                    ph_a2t.append(a2t)
                    ph_p0.append(p0)
                # phase 3: dependent matmuls + normalization per subchunk
                z_subs = []
                for s in range(nsub):
                    a2t, p0 = ph_a2t[s], ph_p0[s]
                    ps_z = pspool.tile([D, 128], F32, tag="ps")
                    nc.tensor.matmul(
                        ps_z[:], lhsT=a2t[:], rhs=p0[:], start=True, stop=True,
                    )
                    ps_d = pspool.tile([1, 128], F32, tag="ps")
                    nc.tensor.matmul(
                        ps_d[:], lhsT=ones_col[:], rhs=p0[:],
                        start=True, stop=True,
                    )
                    r_sb = apool.tile([1, 128], F32, tag="r")
                    nc.vector.reciprocal(r_sb[:], ps_d[:])
                    ps_r = pspool.tile([D, 128], F32, tag="ps")
                    nc.tensor.matmul(
                        ps_r[:], lhsT=ones_sb[:, 0:D], rhs=r_sb[:],
                        start=True, stop=True,
                    )
                    r64_sb = apool.tile([D, 128], F32, tag="r64")
                    nc.scalar.activation(r64_sb[:], ps_r[:], AF.Copy)
                    z_t = apool.tile([D, 4 * SUB], F32, tag="z")
                    nc.vector.tensor_mul(z_t[:], ps_z[:], r64_sb[:])
                    z_subs.append(z_t)

                # GEMM2 + bias; the w-broadcast happens in the matmul rhs
                # (step-0 AP re-reads each z column 4x) so the residual
                # add runs on plain stride-1 APs at full DVE rate.
                nsb = SUB
                for j in range(4):
                    for h in range(nsub):
                        nsl = slice(h * nsb, (h + 1) * nsb)
                        zv = (
                            z_subs[h][:]
                            .rearrange("p (n l) -> p n l", l=4)
                            .unsqueeze(3)
                            .broadcast_to((D, nsb, 4, 4))
                        )
                        ps_y = pypool.tile([128, 16 * nsb], F32, tag="psy")
                        nc.tensor.matmul(
                            ps_y[:], lhsT=w4t_sb[:, j], rhs=zv[:],
                            start=True, stop=True,
                        )
                        nc.vector.scalar_tensor_tensor(
                            xtv[:, nsl, j],
                            ps_y[:].rearrange("p (n h w) -> p n h w", h=4, w=4),
                            b4c_sb[:, j:j + 1],
                            xtv[:, nsl, j],
                            op0=mybir.AluOpType.add,
                            op1=mybir.AluOpType.add,
                        )

                # store on the scalar HWDGE queue to overlap with loads
                nc.scalar.dma_start(ov[b], x_t[:])

    nc.compile()
    _PROG_CACHE[key] = nc
    return nc


def prep_params(W123, b123, g123, be123, m123, v123, W4, b4, g4, be4, m4, v4):
    """Fold BN into the convs; permute channels for the c=4p+j packing."""
    f32 = np.float32
    s123 = (g123 / np.sqrt(v123 + EPS)).astype(f32)            # (3, D)
    Wf = (W123 * s123[:, :, None]).astype(f32)                 # (3, D, C)
    bf = ((b123 - m123) * s123 + be123).astype(f32)            # (3, D)
    s4 = (g4 / np.sqrt(v4 + EPS)).astype(f32)                  # (C,)
    W4f = (W4 * s4[:, None]).astype(f32)                       # (C, D)
    b4f = ((b4 - m4) * s4 + be4).astype(f32)                   # (C,)

    # perm[j*128 + p] = 4p + j : row j*128+p of a device weight tensor
    # holds original channel 4p+j (matching the x packing).
    p_idx, j_idx = np.meshgrid(np.arange(128), np.arange(4), indexing="ij")
    perm = (4 * p_idx + j_idx).T.reshape(-1)                   # (512,)

    wqk = np.concatenate([Wf[0].T, Wf[1].T], axis=1)[perm]     # (C, 128)
    bqk = np.concatenate([bf[0], bf[1]])[:, None]              # (128, 1)
    w2a = np.ascontiguousarray(Wf[2].T[perm])                  # (C, D)
    b2a = bf[2][None, :]                                       # (1, D)
    w4t = np.ascontiguousarray(W4f.T[:, perm])                 # (D, C)
    b4v = b4f[perm][None, :]                                   # (1, C)
    msk = np.kron(np.eye(SUB, dtype=f32), np.ones((4, 4), f32))  # (128, 128)
    return dict(
        wqk=np.ascontiguousarray(wqk), bqk=np.ascontiguousarray(bqk),
        w2a=w2a, b2a=np.ascontiguousarray(b2a),
        w4t=w4t, b4v=np.ascontiguousarray(b4v), msk=msk,
    )


def _run(inputs, trace=False, **spmd_kwargs):
    from concourse.bass_utils import run_bass_kernel_spmd

    x = np.ascontiguousarray(np.asarray(inputs["x"], dtype=np.float32))
    params = prep_params(**{k: np.asarray(v, np.float64)
                            for k, v in inputs.items() if k != "x"})
    nc = build_program()
    in_maps = [
        {"x": x[i * NSH:(i + 1) * NSH], **params} for i in range(NCORES)
    ]
    res = run_bass_kernel_spmd(
        nc, in_maps, list(range(NCORES)), trace=trace, **spmd_kwargs
    )
    outs = np.concatenate(
        [np.asarray(res.results[i]["out"]) for i in range(NCORES)], axis=0
    )
    return outs, res


def kernel(**inputs):
    outs, _ = _run(inputs)
    return outs



# revision 3
# speedup vs baseline: 2.2520x; 2.2520x over previous
"""Trainium2 Bass kernel for the attention-gate block (bf16 pipeline).

Math (per sample n, after folding BN into the convs):
  X     = x[n, :, ::2, ::2].reshape(C, 4)                 # C=512, L=4
  act_k = relu(Wk' @ X + bk')            k=0,1,2          # D=64 each
  S     = act0^T act1  (4x4);  P = softmax_rows(S)
  Z     = P @ act2^T  (4x64)
  Y     = W4' @ Z^T + b4'                                  # (512, 4)
  out[n,c,h,w] = x[n,c,h,w] + Y[c,h]                       # broadcast over w

Design (per core, 256 samples, blocks of 128):
  - everything on the wire and in the matmuls is bf16 (fp32 PSUM accum):
    4x fewer PE cycles than fp32 and half the HBM traffic.  Measured
    numerical impact on the final output is fro ~3e-3 (gate is 2e-2).
  - x is repacked HOST-side to [core][p, n, (j h w)] with c = 4p + j, so
    each block DMA is 128 partitions x 16KB contiguous (vs 128B runs when
    DMAing from the NCHW layout directly) -- line-rate HBM.
  - GEMM1 computes q and k over 4 contraction groups; v is computed
    directly transposed ([samples*4 parts, d]) by swapping matmul
    operands, with its bias folded in via a K=1 ones-row matmul.
  - attention scores for 32 samples at a time come from one [64]x[128,128]
    gram matmul whose block-diagonal 4x4 blocks are the real scores;
    masked exp (ACT + 0/1 block-diag mask on DVE); softmax denominators
    via a ones-column matmul; normalization deferred past the P@V matmul.
  - GEMM2 (BN folded): z for all 4 subchunks is collected in one [64, 4B]
    SBUF tile so conv2 is ONE 512-col matmul per channel group j; the
    w-broadcast happens in the residual STT's PSUM-read AP (stride-0).
"""

import os
import sys

for _p in ("/opt/trn_rl_repo",):
    if _p not in sys.path:
        sys.path.insert(0, _p)

import numpy as np
from ml_dtypes import bfloat16

import concourse.mybir as mybir
from concourse import bacc, tile

EPS = 1e-5
N_TOTAL, C, D, HH, WW = 2048, 512, 64, 4, 4
NCORES = 8
NSH = N_TOTAL // NCORES  # 256 samples per core
BLK = int(os.environ.get("KBLK", "128"))  # samples per block
SUB = 32                 # samples per attention subchunk (4*SUB = 128 cols)
SHIFT = -34.0            # constant exp shift; cancels in the normalization
F32 = mybir.dt.float32
BF16 = mybir.dt.bfloat16

_PROG_CACHE = {}


def build_program(nsh=NSH, blk=BLK, reps=1):
    key = (nsh, blk, reps)
    if key in _PROG_CACHE:
        return _PROG_CACHE[key]

    nc = bacc.Bacc("TRN2", target_bir_lowering=False, debug=False)
    AF = mybir.ActivationFunctionType

    x_in = nc.dram_tensor("x", (128, nsh * 64), BF16, kind="ExternalInput")
    wq = nc.dram_tensor("wq", (128, 4, D), BF16, kind="ExternalInput")
    wk = nc.dram_tensor("wk", (128, 4, D), BF16, kind="ExternalInput")
    w2a = nc.dram_tensor("w2a", (128, 4, D), BF16, kind="ExternalInput")
    w4t = nc.dram_tensor("w4t", (D, 4, 128), BF16, kind="ExternalInput")
    bq = nc.dram_tensor("bq", (D, 1), F32, kind="ExternalInput")
    bk = nc.dram_tensor("bk", (D, 1), F32, kind="ExternalInput")
    b2a = nc.dram_tensor("b2a", (1, D), BF16, kind="ExternalInput")
    b4c = nc.dram_tensor("b4c", (128, 4), F32, kind="ExternalInput")
    msk = nc.dram_tensor("msk", (128, 128), BF16, kind="ExternalInput")
    out = nc.dram_tensor("out", (128, nsh * 64), BF16, kind="ExternalOutput")

    nblk = nsh // blk
    nsub = blk // SUB
    NF = 4 * blk  # free width of a full block of (n, l) columns

    with tile.TileContext(nc) as tc:
        with (
            tc.tile_pool(name="const", bufs=1) as cpool,
            tc.tile_pool(name="xp", bufs=(3 if blk >= 128 else 4)) as xpool,
            tc.tile_pool(name="work", bufs=4) as wpool,
            tc.tile_pool(name="att", bufs=6) as apool,
            tc.tile_pool(name="ps", bufs=6, space="PSUM") as pspool,
            tc.tile_pool(name="psy", bufs=2, space="PSUM") as pypool,
        ):
            wq_sb = cpool.tile([128, 4, D], BF16)
            nc.sync.dma_start(wq_sb[:], wq[:])
            wk_sb = cpool.tile([128, 4, D], BF16)
            nc.sync.dma_start(wk_sb[:], wk[:])
            w2a_sb = cpool.tile([128, 4, D], BF16)
            nc.sync.dma_start(w2a_sb[:], w2a[:])
            w4t_sb = cpool.tile([D, 4, 128], BF16)
            nc.sync.dma_start(w4t_sb[:], w4t[:])
            bq_sb = cpool.tile([D, 1], F32)
            nc.sync.dma_start(bq_sb[:], bq[:])
            bk_sb = cpool.tile([D, 1], F32)
            nc.sync.dma_start(bk_sb[:], bk[:])
            b2a_sb = cpool.tile([1, D], BF16)
            nc.sync.dma_start(b2a_sb[:], b2a[:])
            b4c_sb = cpool.tile([128, 4], F32)
            nc.sync.dma_start(b4c_sb[:], b4c[:])
            msk_sb = cpool.tile([128, 128], BF16)
            nc.sync.dma_start(msk_sb[:], msk[:])
            ones_sb = cpool.tile([1, 128], BF16)
            nc.vector.memset(ones_sb[:], 1.0)
            ones_col = cpool.tile([128, 1], BF16)
            nc.vector.memset(ones_col[:], 1.0)
            shift_sb = cpool.tile([128, 1], F32)
            nc.vector.memset(shift_sb[:], SHIFT)

            xv = x_in[:].rearrange("p (b n f) -> b p n f", n=blk, f=64)
            ov = out[:].rearrange("p (b n f) -> b p n f", n=blk, f=64)

            for b in [b for _ in range(reps) for b in range(nblk)]:
                x_t = xpool.tile([128, blk, 64], BF16, tag="x")
                nc.sync.dma_start(x_t[:], xv[b])
                xtv = x_t[:].rearrange("p n (j h w) -> p n j h w", j=4, h=4)

                # gather the ::2,::2 columns -> [128, j, n, l] with l=(h',w')
                xr = wpool.tile([128, 4, blk, 4], BF16, tag="xr")
                nc.vector.tensor_copy(
                    xr[:].rearrange("p j n (a c) -> p j n a c", a=2),
                    xtv[:, :, :, 0:4:2, 0:4:2].transpose([0, 2, 1, 3, 4]),
                )
                xrf = xr[:].rearrange("p j n l -> p j (n l)")

                # GEMM1 q and k: [c=512 contraction] -> psum [64, NF] each
                ps_q = pspool.tile([D, NF], F32, tag="ps")
                ps_k = pspool.tile([D, NF], F32, tag="ps")
                for j in range(4):
                    nc.tensor.matmul(
                        ps_q[:], lhsT=wq_sb[:, j], rhs=xrf[:, j],
                        start=(j == 0), stop=(j == 3),
                    )
                for j in range(4):
                    nc.tensor.matmul(
                        ps_k[:], lhsT=wk_sb[:, j], rhs=xrf[:, j],
                        start=(j == 0), stop=(j == 3),
                    )
                a_q = wpool.tile([D, NF], BF16, tag="aq")
                nc.scalar.activation(a_q[:], ps_q[:], AF.Relu, bias=bq_sb[:])
                a_k = wpool.tile([D, NF], BF16, tag="ak")
                nc.scalar.activation(a_k[:], ps_k[:], AF.Relu, bias=bk_sb[:])

                # phase 1: independent PE work for all subchunks
                ph_vt, ph_g = [], []
                for s in range(nsub):
                    cl = slice(s * 128, s * 128 + 128)
                    ps_vt = pspool.tile([128, D], F32, tag="ps")
                    for j in range(4):
                        nc.tensor.matmul(
                            ps_vt[:], lhsT=xrf[:, j, cl], rhs=w2a_sb[:, j],
                            start=(j == 0), stop=False,
                        )
                    nc.tensor.matmul(
                        ps_vt[:], lhsT=ones_sb[:], rhs=b2a_sb[:],
                        start=False, stop=True,
                    )
                    ps_g = pspool.tile([128, 128], F32, tag="ps")
                    nc.tensor.matmul(
                        ps_g[:], lhsT=a_k[:, cl], rhs=a_q[:, cl],
                        start=True, stop=True,
                    )
                    ph_vt.append(ps_vt)
                    ph_g.append(ps_g)
                # phase 2: ACT/DVE consumers for all subchunks
                ph_a2t, ph_p0 = [], []
                for s in range(nsub):
                    a2t = apool.tile([128, D], BF16, tag="a2t")
                    nc.scalar.activation(a2t[:], ph_vt[s][:], AF.Relu)
                    e_t = apool.tile([128, 128], BF16, tag="e")
                    nc.scalar.activation(e_t[:], ph_g[s][:], AF.Exp,
                                         bias=shift_sb[:])
                    p0 = apool.tile([128, 128], BF16, tag="p0")
                    nc.vector.tensor_mul(p0[:], e_t[:], msk_sb[:])
                    ph_a2t.append(a2t)
                    ph_p0.append(p0)
                # phase 3: dependent matmuls + normalization per subchunk;
                # normalized z collects into one [D, NF] tile for GEMM2
                z_blk = apool.tile([D, NF], BF16, tag="z")
                for s in range(nsub):
                    a2t, p0 = ph_a2t[s], ph_p0[s]
                    ps_z = pspool.tile([D, 128], F32, tag="ps")
                    nc.tensor.matmul(
                        ps_z[:], lhsT=a2t[:], rhs=p0[:], start=True, stop=True,
                    )
                    ps_d = pspool.tile([1, 128], F32, tag="ps")
                    nc.tensor.matmul(
                        ps_d[:], lhsT=ones_col[:], rhs=p0[:],
                        start=True, stop=True,
                    )
                    r_sb = apool.tile([1, 128], BF16, tag="r")
                    with nc.allow_low_precision("bf16 softmax normalization"):
                        nc.vector.reciprocal(r_sb[:], ps_d[:])
                    ps_r = pspool.tile([D, 128], F32, tag="ps")
                    nc.tensor.matmul(
                        ps_r[:], lhsT=ones_sb[:, 0:D], rhs=r_sb[:],
                        start=True, stop=True,
                    )
                    r64_sb = apool.tile([D, 128], BF16, tag="r64")
                    nc.scalar.activation(r64_sb[:], ps_r[:], AF.Copy)
                    nc.vector.tensor_mul(
                        z_blk[:, s * 128:(s + 1) * 128], ps_z[:], r64_sb[:])

                # GEMM2 + bias + residual: one 512-col matmul per channel
                # group j; the w-broadcast happens in the STT's PSUM-read
                # AP (stride-0 last dim).
                for j in range(4):
                    ps_y = pypool.tile([128, NF], F32, tag="psy")
                    nc.tensor.matmul(
                        ps_y[:], lhsT=w4t_sb[:, j], rhs=z_blk[:],
                        start=True, stop=True,
                    )
                    yv = (
                        ps_y[:]
                        .rearrange("p (n l) -> p n l", l=4)
                        .unsqueeze(3)
                        .broadcast_to((128, blk, 4, 4))
                    )
                    nc.vector.scalar_tensor_tensor(
                        xtv[:, :, j],
                        yv,
                        b4c_sb[:, j:j + 1],
                        xtv[:, :, j],
                        op0=mybir.AluOpType.add,
                        op1=mybir.AluOpType.add,
                    )

                # store on the scalar HWDGE queue to overlap with loads
                nc.scalar.dma_start(ov[b], x_t[:])

    nc.compile()
    _PROG_CACHE[key] = nc
    return nc


def _bf16(a):
    return np.ascontiguousarray(np.asarray(a, np.float32)).astype(bfloat16)


def prep_params(W123, b123, g123, be123, m123, v123, W4, b4, g4, be4, m4, v4):
    """Fold BN into the convs; permute channels for the c=4p+j packing."""
    f64 = np.float64
    s123 = (g123 / np.sqrt(v123 + EPS)).astype(f64)            # (3, D)
    Wf = (W123 * s123[:, :, None]).astype(f64)                 # (3, D, C)
    bf = ((b123 - m123) * s123 + be123).astype(np.float32)     # (3, D)
    s4 = (g4 / np.sqrt(v4 + EPS)).astype(f64)                  # (C,)
    W4f = (W4 * s4[:, None]).astype(f64)                       # (C, D)
    b4f = ((b4 - m4) * s4 + be4).astype(np.float32)            # (C,)

    # perm[j*128 + p] = 4p + j : row j*128+p of a device weight tensor
    # holds original channel 4p+j (matching the x packing).
    p_idx, j_idx = np.meshgrid(np.arange(128), np.arange(4), indexing="ij")
    perm = (4 * p_idx + j_idx).T.reshape(-1)                   # (512,)

    def to_pjd(w):  # (D, C) weights -> [128, 4, D] with c = 4p+j
        return _bf16(w.T[perm].reshape(4, 128, D).transpose(1, 0, 2))

    msk = np.kron(np.eye(SUB, dtype=np.float32), np.ones((4, 4), np.float32))
    return dict(
        wq=to_pjd(Wf[0]), wk=to_pjd(Wf[1]), w2a=to_pjd(Wf[2]),
        w4t=_bf16(W4f.T[:, perm].reshape(D, 4, 128)),
        bq=np.ascontiguousarray(bf[0][:, None]),
        bk=np.ascontiguousarray(bf[1][:, None]),
        b2a=_bf16(bf[2][None, :]),
        b4c=np.ascontiguousarray(b4f[perm].reshape(4, 128).T),
        msk=_bf16(msk),
    )


def pack_x(x):
    """(N, C, 4, 4) fp32 -> [NCORES][128, NSH*64] bf16, c = 4p+j packed."""
    xb = np.asarray(x, np.float32).reshape(NCORES, NSH, 128, 4, 16)
    xb = xb.astype(bfloat16).transpose(0, 2, 1, 3, 4)
    return np.ascontiguousarray(xb).reshape(NCORES, 128, NSH * 64)


def unpack_out(outs):
    """[NCORES][128, NSH*64] bf16 -> (N, C, 4, 4) fp32."""
    o = np.stack([np.asarray(c) for c in outs]).reshape(
        NCORES, 128, NSH, 4, 16).transpose(0, 2, 1, 3, 4)
    return np.ascontiguousarray(o).reshape(N_TOTAL, C, HH, WW).astype(
        np.float32)


def _run(inputs, trace=False, **spmd_kwargs):
    from concourse.bass_utils import run_bass_kernel_spmd

    xp = pack_x(inputs["x"])
    params = prep_params(**{k: np.asarray(v, np.float64)
                            for k, v in inputs.items() if k != "x"})
    nc = build_program()
    in_maps = [{"x": xp[i], **params} for i in range(NCORES)]
    res = run_bass_kernel_spmd(
        nc, in_maps, list(range(NCORES)), trace=trace, **spmd_kwargs
    )
    outs = unpack_out([res.results[i]["out"] for i in range(NCORES)])
    return outs, res


def kernel(**inputs):
    outs, _ = _run(inputs)
    return outs


# revision 4
# speedup vs baseline: 2.9280x; 1.3002x over previous
"""Trainium2 Bass kernel for the attention-gate block (bf16 pipeline).

Math (per sample n, after folding BN into the convs):
  X     = x[n, :, ::2, ::2].reshape(C, 4)                 # C=512, L=4
  act_k = relu(Wk' @ X + bk')            k=0,1,2          # D=64 each
  S     = act0^T act1  (4x4);  P = softmax_rows(S)
  Z     = P @ act2^T  (4x64)
  Y     = W4' @ Z^T + b4'                                  # (512, 4)
  out[n,c,h,w] = x[n,c,h,w] + Y[c,h]                       # broadcast over w

Design (per core, 256 samples, blocks of 128):
  - everything on the wire and in the matmuls is bf16 (fp32 PSUM accum):
    4x fewer PE cycles than fp32 and half the HBM traffic.  Measured
    numerical impact on the final output is fro ~3e-3 (gate is 2e-2).
  - x is repacked HOST-side to [core][p, n, (j w h)] with c = 4p + j, so
    each block DMA is 128 partitions x 16KB contiguous -- line-rate HBM.
    The in-run order (j, w, h) makes the residual add's inner dimension
    (h) match y's layout, so the broadcast-over-w add runs stride-1.
  - GEMM1 computes q and k over 4 contraction groups; v is computed
    directly transposed ([samples*4 parts, d]) by swapping matmul
    operands, with its bias folded in via a K=1 ones-row matmul.
  - attention scores for 32 samples at a time come from one [64]x[128,128]
    gram matmul whose block-diagonal 4x4 blocks are the real scores;
    masked exp (ACT + 0/1 block-diag mask on DVE).  Softmax denominators
    are computed TRANSPOSED (lhsT=p0, rhs=ones -> [128,1] per sub, batched
    into one [128,4] bank) so a single DVE reciprocal runs on 128 lanes
    (the [1,128] orientation costs ~1us/op on one lane).  r is broadcast
    back to [64,128] rows via an identity-rhs matmul with a stride-0 lhsT.
  - GEMM2 (BN folded): z for all 4 subchunks collects into one [64, 4B]
    SBUF tile so conv2 is ONE 512-col matmul per channel group j; the
    conv2 bias is applied by the ACT PSUM->SBUF copy (per-partition bias),
    and the residual is a pure bf16 SBUF tensor_tensor add at DVE 2x mode
    (y read through a stride-0-over-w broadcast AP).
"""

import os
import sys

for _p in ("/opt/trn_rl_repo",):
    if _p not in sys.path:
        sys.path.insert(0, _p)

import numpy as np
from ml_dtypes import bfloat16

import concourse.mybir as mybir
from concourse import bacc, tile

EPS = 1e-5
N_TOTAL, C, D, HH, WW = 2048, 512, 64, 4, 4
NCORES = 8
NSH = N_TOTAL // NCORES  # 256 samples per core
BLK = int(os.environ.get("KBLK", "128"))  # samples per block
SUB = 32                 # samples per attention subchunk (4*SUB = 128 cols)
SHIFT = -34.0            # constant exp shift; cancels in the normalization
F32 = mybir.dt.float32
BF16 = mybir.dt.bfloat16

_PROG_CACHE = {}


def build_program(nsh=NSH, blk=BLK, reps=1):
    key = (nsh, blk, reps)
    if key in _PROG_CACHE:
        return _PROG_CACHE[key]

    nc = bacc.Bacc("TRN2", target_bir_lowering=False, debug=False)
    AF = mybir.ActivationFunctionType

    x_in = nc.dram_tensor("x", (128, nsh * 64), BF16, kind="ExternalInput")
    wq = nc.dram_tensor("wq", (128, 4, D), BF16, kind="ExternalInput")
    wk = nc.dram_tensor("wk", (128, 4, D), BF16, kind="ExternalInput")
    w2a = nc.dram_tensor("w2a", (128, 4, D), BF16, kind="ExternalInput")
    w4t = nc.dram_tensor("w4t", (D, 4, 128), BF16, kind="ExternalInput")
    bq = nc.dram_tensor("bq", (D, 1), F32, kind="ExternalInput")
    bk = nc.dram_tensor("bk", (D, 1), F32, kind="ExternalInput")
    b2a = nc.dram_tensor("b2a", (1, D), BF16, kind="ExternalInput")
    b4c = nc.dram_tensor("b4c", (128, 4), F32, kind="ExternalInput")
    msk = nc.dram_tensor("msk", (128, 128), BF16, kind="ExternalInput")
    eye = nc.dram_tensor("eye", (128, 128), BF16, kind="ExternalInput")
    out = nc.dram_tensor("out", (128, nsh * 64), BF16, kind="ExternalOutput")

    nblk = nsh // blk
    nsub = blk // SUB
    NF = 4 * blk  # free width of a full block of (n, l) columns

    with tile.TileContext(nc) as tc:
        with (
            tc.tile_pool(name="const", bufs=1) as cpool,
            tc.tile_pool(name="xp", bufs=(3 if blk >= 128 else 4)) as xpool,
            tc.tile_pool(name="work", bufs=4) as wpool,
            tc.tile_pool(name="att", bufs=6) as apool,
            tc.tile_pool(name="ps", bufs=5, space="PSUM") as pspool,
            tc.tile_pool(name="psd", bufs=1, space="PSUM") as pdpool,
            tc.tile_pool(name="psy", bufs=2, space="PSUM") as pypool,
        ):
            wq_sb = cpool.tile([128, 4, D], BF16)
            nc.sync.dma_start(wq_sb[:], wq[:])
            wk_sb = cpool.tile([128, 4, D], BF16)
            nc.sync.dma_start(wk_sb[:], wk[:])
            w2a_sb = cpool.tile([128, 4, D], BF16)
            nc.sync.dma_start(w2a_sb[:], w2a[:])
            w4t_sb = cpool.tile([D, 4, 128], BF16)
            nc.sync.dma_start(w4t_sb[:], w4t[:])
            bq_sb = cpool.tile([D, 1], F32)
            nc.sync.dma_start(bq_sb[:], bq[:])
            bk_sb = cpool.tile([D, 1], F32)
            nc.sync.dma_start(bk_sb[:], bk[:])
            b2a_sb = cpool.tile([1, D], BF16)
            nc.sync.dma_start(b2a_sb[:], b2a[:])
            b4c_sb = cpool.tile([128, 4], F32)
            nc.sync.dma_start(b4c_sb[:], b4c[:])
            msk_sb = cpool.tile([128, 128], BF16)
            nc.sync.dma_start(msk_sb[:], msk[:])
            eye_sb = cpool.tile([128, 128], BF16)
            nc.sync.dma_start(eye_sb[:], eye[:])
            ones_sb = cpool.tile([1, 128], BF16)
            nc.vector.memset(ones_sb[:], 1.0)
            ones_col = cpool.tile([128, 1], BF16)
            nc.vector.memset(ones_col[:], 1.0)
            shift_sb = cpool.tile([128, 1], F32)
            nc.vector.memset(shift_sb[:], SHIFT)

            xv = x_in[:].rearrange("p (b n f) -> b p n f", n=blk, f=64)
            ov = out[:].rearrange("p (b n f) -> b p n f", n=blk, f=64)

            for b in [b for _ in range(reps) for b in range(nblk)]:
                x_t = xpool.tile([128, blk, 64], BF16, tag="x")
                nc.sync.dma_start(x_t[:], xv[b])
                # in-run order is (j, w, h)
                xtv = x_t[:].rearrange("p n (j w h) -> p n j w h", j=4, w=4)

                # gather the ::2,::2 columns -> [128, j, n, l], l=(h',w')
                xr = wpool.tile([128, 4, blk, 4], BF16, tag="xr")
                nc.vector.tensor_copy(
                    xr[:].rearrange("p j n (a c) -> p j n a c", a=2),
                    xtv[:, :, :, 0:4:2, 0:4:2].transpose([0, 2, 1, 4, 3]),
                )
                xrf = xr[:].rearrange("p j n l -> p j (n l)")

                # GEMM1 q and k: [c=512 contraction] -> psum [64, NF] each
                ps_q = pspool.tile([D, NF], F32, tag="ps")
                ps_k = pspool.tile([D, NF], F32, tag="ps")
                for j in range(4):
                    nc.tensor.matmul(
                        ps_q[:], lhsT=wq_sb[:, j], rhs=xrf[:, j],
                        start=(j == 0), stop=(j == 3),
                    )
                for j in range(4):
                    nc.tensor.matmul(
                        ps_k[:], lhsT=wk_sb[:, j], rhs=xrf[:, j],
                        start=(j == 0), stop=(j == 3),
                    )
                a_q = wpool.tile([D, NF], BF16, tag="aq")
                nc.scalar.activation(a_q[:], ps_q[:], AF.Relu, bias=bq_sb[:])
                a_k = wpool.tile([D, NF], BF16, tag="ak")
                nc.scalar.activation(a_k[:], ps_k[:], AF.Relu, bias=bk_sb[:])

                # phase 1: independent PE work for all subchunks
                ph_vt, ph_g = [], []
                for s in range(nsub):
                    cl = slice(s * 128, s * 128 + 128)
                    ps_vt = pspool.tile([128, D], F32, tag="ps")
                    for j in range(4):
                        nc.tensor.matmul(
                            ps_vt[:], lhsT=xrf[:, j, cl], rhs=w2a_sb[:, j],
                            start=(j == 0), stop=False,
                        )
                    nc.tensor.matmul(
                        ps_vt[:], lhsT=ones_sb[:], rhs=b2a_sb[:],
                        start=False, stop=True,
                    )
                    ps_g = pspool.tile([128, 128], F32, tag="ps")
                    nc.tensor.matmul(
                        ps_g[:], lhsT=a_k[:, cl], rhs=a_q[:, cl],
                        start=True, stop=True,
                    )
                    ph_vt.append(ps_vt)
                    ph_g.append(ps_g)
                # phase 2: ACT/DVE consumers for all subchunks
                ph_a2t, ph_p0 = [], []
                for s in range(nsub):
                    a2t = apool.tile([128, D], BF16, tag="a2t")
                    nc.scalar.activation(a2t[:], ph_vt[s][:], AF.Relu)
                    e_t = apool.tile([128, 128], BF16, tag="e")
                    nc.scalar.activation(e_t[:], ph_g[s][:], AF.Exp,
                                         bias=shift_sb[:])
                    p0 = apool.tile([128, 128], BF16, tag="p0")
                    nc.vector.tensor_mul(p0[:], e_t[:], msk_sb[:])
                    ph_a2t.append(a2t)
                    ph_p0.append(p0)
                # denominators, transposed: one [128, nsub] PSUM bank so a
                # single reciprocal runs across all 128 lanes
                ps_dt = pdpool.tile([128, nsub], F32, tag="psd")
                for s in range(nsub):
                    nc.tensor.matmul(
                        ps_dt[:, s:s + 1], lhsT=ph_p0[s][:], rhs=ones_col[:],
                        start=True, stop=True,
                    )
                rt_sb = apool.tile([128, nsub], BF16, tag="rt")
                with nc.allow_low_precision("bf16 softmax normalization"):
                    nc.vector.reciprocal(rt_sb[:], ps_dt[:])
                # phase 3: dependent matmuls + normalization per subchunk;
                # normalized z collects into one [D, NF] tile for GEMM2
                z_blk = apool.tile([D, NF], BF16, tag="z")
                for s in range(nsub):
                    a2t, p0 = ph_a2t[s], ph_p0[s]
                    ps_z = pspool.tile([D, 128], F32, tag="ps")
                    nc.tensor.matmul(
                        ps_z[:], lhsT=a2t[:], rhs=p0[:], start=True, stop=True,
                    )
                    # broadcast r to [64, 128]: rT column (stride-0 over d)
                    # as stationary against an identity rhs
                    ps_r = pspool.tile([D, 128], F32, tag="ps")
                    nc.tensor.matmul(
                        ps_r[:],
                        lhsT=rt_sb[:, s:s + 1].broadcast_to((128, D)),
                        rhs=eye_sb[:],
                        start=True, stop=True,
                    )
                    r64_sb = apool.tile([D, 128], BF16, tag="r64")
                    nc.scalar.activation(r64_sb[:], ps_r[:], AF.Copy)
                    nc.vector.tensor_mul(
                        z_blk[:, s * 128:(s + 1) * 128], ps_z[:], r64_sb[:])

                # GEMM2: one 512-col matmul per channel group j; bias via
                # the ACT PSUM->SBUF copy; residual add at DVE 2x (bf16
                # SBUF, stride-0-over-w broadcast read of y)
                for j in range(4):
                    ps_y = pypool.tile([128, NF], F32, tag="psy")
                    nc.tensor.matmul(
                        ps_y[:], lhsT=w4t_sb[:, j], rhs=z_blk[:],
                        start=True, stop=True,
                    )
                    y_sb = apool.tile([128, NF], BF16, tag="y")
                    nc.scalar.activation(y_sb[:], ps_y[:], AF.Identity,
                                         bias=b4c_sb[:, j:j + 1])
                    yv = (
                        y_sb[:]
                        .rearrange("p (n l) -> p n l", l=4)
                        .unsqueeze(2)
                        .broadcast_to((128, blk, 4, 4))
                    )
                    nc.vector.tensor_add(xtv[:, :, j], yv, xtv[:, :, j])

                # store on the scalar HWDGE queue to overlap with loads
                nc.scalar.dma_start(ov[b], x_t[:])

    nc.compile()
    _PROG_CACHE[key] = nc
    return nc


def _bf16(a):
    return np.ascontiguousarray(np.asarray(a, np.float32)).astype(bfloat16)


def prep_params(W123, b123, g123, be123, m123, v123, W4, b4, g4, be4, m4, v4):
    """Fold BN into the convs; permute channels for the c=4p+j packing."""
    f64 = np.float64
    s123 = (g123 / np.sqrt(v123 + EPS)).astype(f64)            # (3, D)
    Wf = (W123 * s123[:, :, None]).astype(f64)                 # (3, D, C)
    bf = ((b123 - m123) * s123 + be123).astype(np.float32)     # (3, D)
    s4 = (g4 / np.sqrt(v4 + EPS)).astype(f64)                  # (C,)
    W4f = (W4 * s4[:, None]).astype(f64)                       # (C, D)
    b4f = ((b4 - m4) * s4 + be4).astype(np.float32)            # (C,)

    # perm[j*128 + p] = 4p + j : row j*128+p of a device weight tensor
    # holds original channel 4p+j (matching the x packing).
    p_idx, j_idx = np.meshgrid(np.arange(128), np.arange(4), indexing="ij")
    perm = (4 * p_idx + j_idx).T.reshape(-1)                   # (512,)

    def to_pjd(w):  # (D, C) weights -> [128, 4, D] with c = 4p+j
        return _bf16(w.T[perm].reshape(4, 128, D).transpose(1, 0, 2))

    msk = np.kron(np.eye(SUB, dtype=np.float32), np.ones((4, 4), np.float32))
    return dict(
        wq=to_pjd(Wf[0]), wk=to_pjd(Wf[1]), w2a=to_pjd(Wf[2]),
        w4t=_bf16(W4f.T[:, perm].reshape(D, 4, 128)),
        bq=np.ascontiguousarray(bf[0][:, None]),
        bk=np.ascontiguousarray(bf[1][:, None]),
        b2a=_bf16(bf[2][None, :]),
        b4c=np.ascontiguousarray(b4f[perm].reshape(4, 128).T),
        msk=_bf16(msk),
        eye=_bf16(np.eye(128, dtype=np.float32)),
    )


def pack_x(x):
    """(N, C, 4, 4) fp32 -> [NCORES][128, NSH*64] bf16.

    c = 4p+j on partitions; per-(p, n) run holds (j, w, h)."""
    xb = np.asarray(x, np.float32).reshape(NCORES, NSH, 128, 4, 4, 4)
    xb = xb.astype(bfloat16).transpose(0, 2, 1, 3, 5, 4)  # core p n j w h
    return np.ascontiguousarray(xb).reshape(NCORES, 128, NSH * 64)


def unpack_out(outs):
    """[NCORES][128, NSH*64] bf16 -> (N, C, 4, 4) fp32."""
    o = np.stack([np.asarray(c) for c in outs]).reshape(
        NCORES, 128, NSH, 4, 4, 4)               # core p n j w h
    o = o.transpose(0, 2, 1, 3, 5, 4)            # core n p j h w
    return np.ascontiguousarray(o).reshape(N_TOTAL, C, HH, WW).astype(
        np.float32)


def _run(inputs, trace=False, **spmd_kwargs):
    from concourse.bass_utils import run_bass_kernel_spmd

    xp = pack_x(inputs["x"])
    params = prep_params(**{k: np.asarray(v, np.float64)
                            for k, v in inputs.items() if k != "x"})
    nc = build_program()
    in_maps = [{"x": xp[i], **params} for i in range(NCORES)]
    res = run_bass_kernel_spmd(
        nc, in_maps, list(range(NCORES)), trace=trace, **spmd_kwargs
    )
    outs = unpack_out([res.results[i]["out"] for i in range(NCORES)])
    return outs, res


def kernel(**inputs):
    outs, _ = _run(inputs)
    return outs
